# revision 16
# baseline (speedup 1.0000x reference)
"""Correlation cost-volume kernel (max_displacement=4) for 8 Trainium2 cores.

Problem: in1, in2: [B=8, C=256, H=128, W=128] f32.
out[b, dy*9+dx, h, w] = sum_c in1[b,c,h,w] * pad(in2)[b, c, h+dy, w+dx]
(pad = 4 zeros on each spatial side), output [8, 81, 128, 128] f32.

Strategy (data-parallel, one batch sample per core):
  For each output row h and each dy (9 values), the TensorEngine computes the
  row-gram  G[w, w'] = sum_c in1[c,h,w] * in2pad[c,h+dy,w']  ([128 x 136],
  contracting C=256 as two K=128 tiles accumulated in PSUM f32; operands are
  bf16, N=408 per matmul).  The 9 needed entries per w are the near-diagonals
  G[w, w+dx], dx in 0..8 — a shear no partition-uniform AP can express.  But
  gpsimd indirect_copy gathers with per-16-partition-core index lists, so each
  core g extracts the 32-wide block-diagonal window starting at
  wstart(g) = min(16g, 104) that covers the 9-diagonals of its 16 partitions.
  This cuts the output DMA from the full 136-wide gram (40 MB/sample) to
  32-wide strips (9.4 MB/sample), un-bottlenecking the DMA (was 98% busy).
  One index per (h, dy) (INNER=32) keeps the gather's per-index RD_CMD cost
  (~50-100 cyc each, ReadOverlap=0 on cayman) off the critical path; the ISA
  dst<=1024-elem limit then allows 3 h rows per gather (43 ragged blocks).
  The host peels the final 9-diagonals from the strips with stride tricks.

  in2 is loaded in 5 overlapping row-chunks per kt (halo 8 = GD*NG-1) issued
  just-in-time, so the first matmul waits on a ~1.2MB chunk instead of the
  whole 8.9MB sample (was a 33us startup stall); the overlap keeps every
  matmul's rhs inside one chunk => one DMA sem-wait per matmul (walrus limit).
  in1 loads are issued one 8-row block ahead so they are never queued behind
  a strip-store's gather-wait on the sync queue (head-of-line blocking).
  NOTE: keep per-partition tile byte sizes 32B-aligned — a 2-byte change to
  the idx tile once shifted every later tile and uniformly slowed the PE ~20%.
"""

import ml_dtypes
import numpy as np

import concourse.bass as bass
import concourse.bacc as bacc
import concourse.mybir as mybir
from concourse.bass_utils import run_bass_kernel_spmd
from concourse.tile import TileContext

B, C, H, W = 8, 256, 128, 128
D = 4
ND = 2 * D + 1  # 9 displacements per axis
WP = W + 2 * D  # 136 padded width / # of gram columns
KT = C // 128  # 2 contraction tiles
GD = 3  # dy rows per PSUM bank (N = 3*136 = 408 <= 512)
NG = ND // GD  # 3 PSUM banks per output row
AH = 8  # h rows per in1 load DMA

INNER = 32  # gather elems per index (ISA: power of 2, <= 32); strip width
NH = 3  # h rows per gather block (dst 3*9*32 = 864 <= 1024 ISA limit)
# ragged blocks: 42 x 3 rows + 2 x 1 row = 128 (1-row tail blocks drain fast)
BLOCKS = [(3 * i, 3) for i in range(42)] + [(126, 1), (127, 1)]
# gram columns are trimmed to the W=128 valid in2 columns (the left/right
# zero-pad columns produce known zeros, filled by the host); the per-core
# 32-wide window starts in trimmed coordinates:
WSTART = [max(0, min(16 * g - D, W - INNER)) for g in range(8)]

# in2 row-chunk boundaries (h ranges); chunk c holds rows [CB[c], CB[c+1]+8)
CB = [0, 10, 34, 66, 98, 128]
NCH = len(CB) - 1
CH_ISSUE = {2: 1, 20: 2, 52: 3, 84: 4}  # h at which to issue chunk c's DMAs

_CACHED_NC = None


def _build_idx(rows: int) -> np.ndarray:
    """Wrapped per-core index lists: one index per (row, dy)."""
    ni = rows * ND
    nw = (ni + 15) // 16
    idx = np.zeros((128, nw), np.uint16)
    for g in range(8):
        for i in range(ni):
            s, p = divmod(i, 16)
            idx[16 * g + p, s] = i * W + WSTART[g]
    return idx


def _chunk_of(h: int) -> int:
    for c in range(NCH):
        if h < CB[c + 1]:
            return c
    raise AssertionError


def _build_nc():
    bf16 = mybir.dt.bfloat16

    nc = bacc.Bacc()
    # in1 as [c][h][kt][w]; in2 zero-padded to [kt][c][hp][wp]; both bf16
    in1_t = nc.declare_dram_parameter("in1_t", [128, H, KT, W], bf16, isOutput=False)
    in2_p = nc.declare_dram_parameter("in2_p", [KT, 128, WP, W], bf16, isOutput=False)
    idx_d = nc.declare_dram_parameter("idxs", [128, 16], mybir.dt.uint16, isOutput=False)
    # 32-wide block-diagonal strips; partition (w) writes contiguous runs;
    # ragged gather blocks concatenate exactly to [w, h, dy, INNER]
    out_g = nc.declare_dram_parameter("out_g", [W, H * ND, INNER], bf16, isOutput=True)

    with TileContext(nc) as tc:
        with (
            tc.tile_pool(name="bpool", bufs=1) as bpool,
            tc.tile_pool(name="apool", bufs=4) as apool,
            tc.tile_pool(name="spool", bufs=6) as spool,
            tc.tile_pool(name="gpool", bufs=16) as gpool,
            tc.tile_pool(name="psum", bufs=2, space="PSUM") as ppool,
        ):
            idx_t = bpool.tile([128, 16], mybir.dt.uint16, name="idxs")
            idx_s = {3: idx_t[:, 0:2], 2: idx_t[:, 2:4], 1: idx_t[:, 4:5]}

            # in2 chunks: [kt][c] -> [128, rows, WP], rows = CB[c+1]+8-CB[c]
            b_cs = [
                [
                    bpool.tile([128, CB[c + 1] + 2 * D - CB[c], W], bf16, name=f"b{kt}c{c}")
                    for c in range(NCH)
                ]
                for kt in range(KT)
            ]

            def load_chunk(c):
                for kt in range(KT):
                    nc.sync.dma_start(
                        out=b_cs[kt][c],
                        in_=in2_p[kt, :, CB[c] : CB[c + 1] + 2 * D],
                    )

            a_tiles = {}

            def load_a(k):
                if k * AH >= H or k in a_tiles:
                    return
                t = apool.tile([128, AH, KT, W], bf16)
                nc.sync.dma_start(out=t, in_=in1_t[:, k * AH : (k + 1) * AH])
                a_tiles[k] = t

            a0 = apool.tile([128, AH, KT, W], bf16)
            nc.sync.dma_start(
                out=b_cs[0][0], in_=in2_p[0, :, CB[0] : CB[1] + 2 * D]
            )
            nc.sync.dma_start(out=a0[:, 0 : AH // 2], in_=in1_t[:, 0 : AH // 2])
            nc.sync.dma_start(
                out=b_cs[1][0], in_=in2_p[1, :, CB[0] : CB[1] + 2 * D]
            )
            nc.sync.dma_start(out=a0[:, AH // 2 : AH], in_=in1_t[:, AH // 2 : AH])
            a_tiles[0] = a0
            load_a(1)
            nc.sync.dma_start(out=idx_t, in_=idx_d[:])

            bi = 0  # current gather block
            st = stv = None
            for h in range(H):
                if h in CH_ISSUE:
                    load_chunk(CH_ISSUE[h])
                if h % AH == 0:
                    load_a(h // AH + 1)
                    a_t = a_tiles.pop(h // AH)

                h0, rows = BLOCKS[bi]
                if h == h0:
                    st = spool.tile([128, rows * ND * W], bf16)
                    stv = st.rearrange("w (a b c) -> w a b c", a=rows, b=ND)

                c = _chunk_of(h)
                loc = h - CB[c]
                pss = [
                    ppool.tile([128, GD * W], mybir.dt.float32, name=f"ps{g}", tag=f"ps{g}")
                    for g in range(NG)
                ]
                skip_g = 0 if h < 2 else (2 if h >= 126 else -1)
                for kt in range(KT):
                    lhsT = a_t[:, h % AH, kt, :]
                    for g in range(NG):
                        if g == skip_g:
                            continue
                        rhs = b_cs[kt][c][:, loc + GD * g : loc + GD * g + GD, :]
                        nc.tensor.matmul(
                            pss[g],
                            lhsT,
                            rhs,
                            start=(kt == 0),
                            stop=(kt == KT - 1),
                        )
                for g in range(NG):
                    if g == skip_g:
                        continue
                    nc.any.tensor_copy(
                        stv[:, h - h0, GD * g : GD * g + GD, :],
                        pss[g].rearrange("w (d p) -> w d p", d=GD),
                    )
                if h == h0 + rows - 1:
                    strip = gpool.tile([128, rows * ND, INNER], bf16)
                    nc.gpsimd.indirect_copy(
                        strip,
                        st.rearrange("w (x e) -> w x e", e=INNER),
                        idx_s[rows],
                        True,
                    )
                    nc.sync.dma_start(
                        out=out_g[:, h0 * ND : (h0 + rows) * ND].rearrange(
                            "w a e -> w (a e)"
                        ),
                        in_=strip.rearrange("w a e -> w (a e)"),
                    )
                    bi += 1

    # Run the bacc passes (move_matmul_waits_to_ldweights /
    # generate_event_semaphores) that enforce the 1-wait-per-instruction HW
    # constraint.  The native run path calls this inside run_bass_kernel_spmd;
    # the axon/bass2jax path serializes nc without it and walrus then rejects
    # matmuls carrying two sync waits.
    nc.compile()
    return nc


def _get_nc():
    global _CACHED_NC
    if _CACHED_NC is None:
        _CACHED_NC = _build_nc()
    return _CACHED_NC


def _make_in_maps(in1: np.ndarray, in2: np.ndarray):
    idx = np.concatenate(
        [_build_idx(3), _build_idx(2), _build_idx(1), np.zeros((128, 11), np.uint16)],
        axis=1,
    )
    in_maps = []
    for b in range(B):
        # [C,H,W] -> [c(128), H, kt, W] so one DMA per h-block is contiguous
        a = np.ascontiguousarray(
            in1[b].astype(ml_dtypes.bfloat16).reshape(KT, 128, H, W).transpose(1, 2, 0, 3)
        )
        p = np.zeros((C, WP, W), ml_dtypes.bfloat16)
        p[:, D : D + H, :] = in2[b].astype(ml_dtypes.bfloat16)
        in_maps.append(
            {"in1_t": a, "in2_p": p.reshape(KT, 128, WP, W), "idxs": idx}
        )
    return in_maps


def _extract_band(g: np.ndarray) -> np.ndarray:
    """[W, H*ND, INNER] strips -> [81, H, W] cost volume.

    g[w, h*ND+dy, e] = G[h, dy][w_trimmed = WSTART[w//16] + e]  (trimmed coords
    = original w' - D).  band[dy*9+dx, h, w] = G at trimmed col w + dx - D,
    which is zero (pad) when w+dx < D or w+dx >= D+W.
    """
    s32 = g.reshape(W, H, ND, INNER).astype(np.float32)  # [w, h, dy, 32]
    iw = np.arange(W)
    pos = (iw - D - np.array(WSTART)[iw // 16])[:, None] + np.arange(ND)[None, :]
    valid = (pos >= 0) & (pos < INNER)  # [w, dx]
    posc = np.clip(pos, 0, INNER - 1)
    band = np.take_along_axis(s32, posc[:, None, None, :], axis=3)  # [w,h,dy,dx]
    band *= valid[:, None, None, :]
    r = np.arange(H)[None, :, None, None] + np.arange(ND)[None, None, :, None]
    band *= (r >= D) & (r < D + H)
    return np.ascontiguousarray(band.transpose(2, 3, 1, 0)).reshape(ND * ND, H, W)


def kernel(**inputs) -> np.ndarray:
    in1 = np.ascontiguousarray(np.asarray(inputs["in1"], dtype=np.float32))
    in2 = np.ascontiguousarray(np.asarray(inputs["in2"], dtype=np.float32))
    assert in1.shape == (B, C, H, W) and in2.shape == (B, C, H, W)

    nc = _get_nc()
    in_maps = _make_in_maps(in1, in2)
    res = run_bass_kernel_spmd(nc, in_maps, list(range(B)))

    outs = [_extract_band(np.asarray(res.results[b]["out_g"])) for b in range(B)]
    return np.stack(outs).astype(np.float32)


# revision 17
# speedup vs baseline: 1.1795x; 1.1795x over previous
"""Correlation cost-volume kernel (max_displacement=4) for 8 Trainium2 cores.

Problem: in1, in2: [B=8, C=256, H=128, W=128] f32.
out[b, dy*9+dx, h, w] = sum_c in1[b,c,h,w] * pad(in2)[b, c, h+dy, w+dx]
(pad = 4 zeros on each spatial side), output [8, 81, 128, 128] f32.

Strategy (data-parallel, one batch sample per core):
  For each output row h and each dy (9 values), the TensorEngine computes the
  row-gram  G[w, w'] = sum_c in1[c,h,w] * in2pad[c,h+dy,w']  ([128 x 136],
  contracting C=256 as two K=128 tiles accumulated in PSUM f32; operands are
  bf16, N=408 per matmul).  The 9 needed entries per w are the near-diagonals
  G[w, w+dx], dx in 0..8 — a shear no partition-uniform AP can express.  But
  gpsimd indirect_copy gathers with per-16-partition-core index lists, so each
  core g extracts the 32-wide block-diagonal window starting at
  wstart(g) = min(16g, 104) that covers the 9-diagonals of its 16 partitions.
  This cuts the output DMA from the full 136-wide gram (40 MB/sample) to
  32-wide strips (9.4 MB/sample), un-bottlenecking the DMA (was 98% busy).
  One index per (h, dy) (INNER=32) keeps the gather's per-index RD_CMD cost
  (~50-100 cyc each, ReadOverlap=0 on cayman) off the critical path; the ISA
  dst<=1024-elem limit then allows 3 h rows per gather (43 ragged blocks).
  The host peels the final 9-diagonals from the strips with stride tricks.

  in2 is loaded in 5 overlapping row-chunks per kt (halo 8 = GD*NG-1) issued
  just-in-time, so the first matmul waits on a ~1.2MB chunk instead of the
  whole 8.9MB sample (was a 33us startup stall); the overlap keeps every
  matmul's rhs inside one chunk => one DMA sem-wait per matmul (walrus limit).
  in1 loads are issued one 8-row block ahead so they are never queued behind
  a strip-store's gather-wait on the sync queue (head-of-line blocking).
  NOTE: keep per-partition tile byte sizes 32B-aligned — a 2-byte change to
  the idx tile once shifted every later tile and uniformly slowed the PE ~20%.
"""

import ml_dtypes
import numpy as np

import concourse.bass as bass
import concourse.bacc as bacc
import concourse.mybir as mybir
from concourse.bass_utils import run_bass_kernel_spmd
from concourse.tile import TileContext

B, C, H, W = 8, 256, 128, 128
D = 4
ND = 2 * D + 1  # 9 displacements per axis
WP = W + 2 * D  # 136 padded width / # of gram columns
KT = C // 128  # 2 contraction tiles
GD = 3  # dy rows per PSUM bank (N = 3*136 = 408 <= 512)
NG = ND // GD  # 3 PSUM banks per output row
AH = 8  # h rows per in1 load DMA

INNER = 32  # gather elems per index (ISA: power of 2, <= 32); strip width
NH = 3  # h rows per gather block (dst 3*9*32 = 864 <= 1024 ISA limit)
# ragged blocks: 42 x 3 rows + 2 x 1 row = 128 (1-row tail blocks drain fast)
BLOCKS = [(3 * i, 3) for i in range(42)] + [(126, 1), (127, 1)]
# gram columns are trimmed to the W=128 valid in2 columns (the left/right
# zero-pad columns produce known zeros, filled by the host); the per-core
# 32-wide window starts in trimmed coordinates:
WSTART = [max(0, min(16 * g - D, W - INNER)) for g in range(8)]

# in2 row-chunk boundaries (h ranges); chunk c holds rows [CB[c], CB[c+1]+8)
CB = [0, 10, 34, 66, 98, 128]
NCH = len(CB) - 1
CH_ISSUE = {2: 1, 20: 2, 52: 3, 84: 4}  # h at which to issue chunk c's DMAs

_CACHED_NC = None


def _build_idx(rows: int) -> np.ndarray:
    """Wrapped per-core index lists: one index per (row, dy)."""
    ni = rows * ND
    nw = (ni + 15) // 16
    idx = np.zeros((128, nw), np.uint16)
    for g in range(8):
        for i in range(ni):
            s, p = divmod(i, 16)
            idx[16 * g + p, s] = i * W + WSTART[g]
    return idx


def _chunk_of(h: int) -> int:
    for c in range(NCH):
        if h < CB[c + 1]:
            return c
    raise AssertionError


def _build_nc():
    bf16 = mybir.dt.bfloat16

    nc = bacc.Bacc()
    # in1 as [c][h][kt][w]; in2 zero-padded to [kt][c][hp][wp]; both bf16
    in1_t = nc.declare_dram_parameter("in1_t", [128, H, KT, W], bf16, isOutput=False)
    in2_p = nc.declare_dram_parameter("in2_p", [KT, 128, WP, W], bf16, isOutput=False)
    idx_d = nc.declare_dram_parameter("idxs", [128, 16], mybir.dt.uint16, isOutput=False)
    # 32-wide block-diagonal strips; partition (w) writes contiguous runs;
    # ragged gather blocks concatenate exactly to [w, h, dy, INNER]
    out_g = nc.declare_dram_parameter("out_g", [W, H * ND, INNER], bf16, isOutput=True)

    with TileContext(nc) as tc:
        with (
            tc.tile_pool(name="bpool", bufs=1) as bpool,
            tc.tile_pool(name="apool", bufs=4) as apool,
            tc.tile_pool(name="spool", bufs=6) as spool,
            tc.tile_pool(name="gpool", bufs=16) as gpool,
            tc.tile_pool(name="psum", bufs=2, space="PSUM") as ppool,
        ):
            idx_t = bpool.tile([128, 16], mybir.dt.uint16, name="idxs")
            idx_s = {3: idx_t[:, 0:2], 2: idx_t[:, 2:4], 1: idx_t[:, 4:5]}

            # in2 chunks: [kt][c] -> [128, rows, WP], rows = CB[c+1]+8-CB[c]
            b_cs = [
                [
                    bpool.tile([128, CB[c + 1] + 2 * D - CB[c], W], bf16, name=f"b{kt}c{c}")
                    for c in range(NCH)
                ]
                for kt in range(KT)
            ]

            def load_chunk(c):
                for kt in range(KT):
                    nc.sync.dma_start(
                        out=b_cs[kt][c],
                        in_=in2_p[kt, :, CB[c] : CB[c + 1] + 2 * D],
                    )

            a_tiles = {}

            def load_a(k):
                if k * AH >= H or k in a_tiles:
                    return
                t = apool.tile([128, AH, KT, W], bf16)
                nc.sync.dma_start(out=t, in_=in1_t[:, k * AH : (k + 1) * AH])
                a_tiles[k] = t

            a0 = apool.tile([128, AH, KT, W], bf16)
            nc.sync.dma_start(out=a0[:, 0 : AH // 2], in_=in1_t[:, 0 : AH // 2])
            load_chunk(0)
            nc.sync.dma_start(out=a0[:, AH // 2 : AH], in_=in1_t[:, AH // 2 : AH])
            a_tiles[0] = a0
            load_a(1)
            nc.sync.dma_start(out=idx_t, in_=idx_d[:])

            bi = 0  # current gather block
            st = stv = None
            for h in range(H):
                if h in CH_ISSUE:
                    load_chunk(CH_ISSUE[h])
                if h % AH == 0:
                    load_a(h // AH + 1)
                    a_t = a_tiles.pop(h // AH)

                h0, rows = BLOCKS[bi]
                if h == h0:
                    st = spool.tile([128, rows * ND * W], bf16)
                    stv = st.rearrange("w (a b c) -> w a b c", a=rows, b=ND)

                c = _chunk_of(h)
                loc = h - CB[c]
                pss = [
                    ppool.tile([128, GD * W], mybir.dt.float32, name=f"ps{g}", tag=f"ps{g}")
                    for g in range(NG)
                ]
                skip_g = 0 if h < 2 else (2 if h >= 126 else -1)
                for kt in range(KT):
                    lhsT = a_t[:, h % AH, kt, :]
                    for g in range(NG):
                        if g == skip_g:
                            continue
                        rhs = b_cs[kt][c][:, loc + GD * g : loc + GD * g + GD, :]
                        nc.tensor.matmul(
                            pss[g],
                            lhsT,
                            rhs,
                            start=(kt == 0),
                            stop=(kt == KT - 1),
                        )
                for g in range(NG):
                    if g == skip_g:
                        continue
                    nc.any.tensor_copy(
                        stv[:, h - h0, GD * g : GD * g + GD, :],
                        pss[g].rearrange("w (d p) -> w d p", d=GD),
                    )
                if h == h0 + rows - 1:
                    strip = gpool.tile([128, rows * ND, INNER], bf16)
                    nc.gpsimd.indirect_copy(
                        strip,
                        st.rearrange("w (x e) -> w x e", e=INNER),
                        idx_s[rows],
                        True,
                    )
                    nc.sync.dma_start(
                        out=out_g[:, h0 * ND : (h0 + rows) * ND].rearrange(
                            "w a e -> w (a e)"
                        ),
                        in_=strip.rearrange("w a e -> w (a e)"),
                    )
                    bi += 1

    # Run the bacc passes (move_matmul_waits_to_ldweights /
    # generate_event_semaphores) that enforce the 1-wait-per-instruction HW
    # constraint.  The native run path calls this inside run_bass_kernel_spmd;
    # the axon/bass2jax path serializes nc without it and walrus then rejects
    # matmuls carrying two sync waits.
    nc.compile()
    return nc


def _get_nc():
    global _CACHED_NC
    if _CACHED_NC is None:
        _CACHED_NC = _build_nc()
    return _CACHED_NC


def _make_in_maps(in1: np.ndarray, in2: np.ndarray):
    idx = np.concatenate(
        [_build_idx(3), _build_idx(2), _build_idx(1), np.zeros((128, 11), np.uint16)],
        axis=1,
    )
    in_maps = []
    for b in range(B):
        # [C,H,W] -> [c(128), H, kt, W] so one DMA per h-block is contiguous
        a = np.ascontiguousarray(
            in1[b].astype(ml_dtypes.bfloat16).reshape(KT, 128, H, W).transpose(1, 2, 0, 3)
        )
        p = np.zeros((C, WP, W), ml_dtypes.bfloat16)
        p[:, D : D + H, :] = in2[b].astype(ml_dtypes.bfloat16)
        in_maps.append(
            {"in1_t": a, "in2_p": p.reshape(KT, 128, WP, W), "idxs": idx}
        )
    return in_maps


def _extract_band(g: np.ndarray) -> np.ndarray:
    """[W, H*ND, INNER] strips -> [81, H, W] cost volume.

    g[w, h*ND+dy, e] = G[h, dy][w_trimmed = WSTART[w//16] + e]  (trimmed coords
    = original w' - D).  band[dy*9+dx, h, w] = G at trimmed col w + dx - D,
    which is zero (pad) when w+dx < D or w+dx >= D+W.
    """
    s32 = g.reshape(W, H, ND, INNER).astype(np.float32)  # [w, h, dy, 32]
    iw = np.arange(W)
    pos = (iw - D - np.array(WSTART)[iw // 16])[:, None] + np.arange(ND)[None, :]
    valid = (pos >= 0) & (pos < INNER)  # [w, dx]
    posc = np.clip(pos, 0, INNER - 1)
    band = np.take_along_axis(s32, posc[:, None, None, :], axis=3)  # [w,h,dy,dx]
    band *= valid[:, None, None, :]
    r = np.arange(H)[None, :, None, None] + np.arange(ND)[None, None, :, None]
    band *= (r >= D) & (r < D + H)
    return np.ascontiguousarray(band.transpose(2, 3, 1, 0)).reshape(ND * ND, H, W)


def kernel(**inputs) -> np.ndarray:
    in1 = np.ascontiguousarray(np.asarray(inputs["in1"], dtype=np.float32))
    in2 = np.ascontiguousarray(np.asarray(inputs["in2"], dtype=np.float32))
    assert in1.shape == (B, C, H, W) and in2.shape == (B, C, H, W)

    nc = _get_nc()
    in_maps = _make_in_maps(in1, in2)
    res = run_bass_kernel_spmd(nc, in_maps, list(range(B)))

    outs = [_extract_band(np.asarray(res.results[b]["out_g"])) for b in range(B)]
    return np.stack(outs).astype(np.float32)
